# revision 26
# baseline (speedup 1.0000x reference)
"""Trainium2 Bass kernel for nn_MessageFunctionForEvent (GNN message function).

Math: the reference is
    em  = W_e2m @ e_wv[b] + b_e2m          (per-node Linear on edge features)
    nw  = W_n2m @ h_w[b]  + b_n2m          (per-node Linear on node features)
    nv  = W_n2m @ h_v[b]  + b_n2m          (node-level, no n axis)
    msg = Wa @ em + Wb @ nw + (Wc @ nv + b_resize)[:, None]
which collapses (precomposing the tiny 128x128 weights on host) to
    msg[b, :, n] = A @ e_wv[b, :, n] + Bm @ h_w[b, :, n] + c[b]
with A = Wa@W_e2m, Bm = Wb@W_n2m, c[b] = Wa@b_e2m + Wb@b_n2m + Wc@nv[b] + b_resize.

The problem is HBM-bound (per-core traffic >> compute), so the streams are
cast to bf16 on the host: e/h chunks and the two 128x128 weights go over the
wire in bf16, matmuls accumulate in fp32 PSUM, the bias-add writes a bf16
output tile, and the host upcasts the result to fp32. This halves HBM traffic
(61.4MB -> 30.7MB per core) for ~1.3e-3 normed rel error (gate is 2e-2).

Device kernel: a single HWDGE ring tops out ~385 GB/s but both rings
together sustain ~425 GB/s (the SBUF AXI fabric limit), so the two rings
are byte-balanced end-to-end: e chunks on the sync(SP) ring, h chunks on
the scalar(ACT) ring, and each chunk's two output halves split across
both rings (ring order alternating per chunk). The packed precomposed
weights (AT|BT|c bit-cast into one bf16 tensor, 520B lines) go over the
sync ring first so compute unblocks immediately; three separate small
weight DMAs would clog the ring for ~6us with sub-512B descriptors.
Per chunk: 512-col bank-aligned matmul segments accumulate A@e then B@h
into two 4-bank fp32 PSUM tiles (all A-segments first — they only need
e — then B as h lands); each half-chunk gets ONE bias-add instruction
(PSUM fp32 -> SBUF bf16), part0 on the VectorE, part1 concurrently on
the ACT engine — one engine alone (~1.1ns/elem/lane) cannot keep up,
especially under HW activity throttling. Load triggers run LOOK=3
chunks ahead of compute/store triggers so an output trigger waiting on
its bias-add never starves a ring (deeper lookahead queues outputs
behind too many input transfers in the ring FIFO and blows up the
drain). The first chunk is split small so the output stream starts
~3us earlier, and the last chunk tapers down so the pipeline drains
with small quanta. Sharding: batch axis (16 batches -> 2 per core).
"""

import sys

import numpy as np
import ml_dtypes

try:
    from concourse import bacc, mybir
except ImportError:  # bare environment: fall back to the in-container repo
    sys.path.append("/opt/trn_rl_repo")
    from concourse import bacc, mybir
import concourse.tile as tile
from concourse.bass_utils import run_bass_kernel_spmd

B, F, N = 16, 128, 20000
NCORES = 8
BPC = B // NCORES          # batches per core
CH = 4000                  # columns per DMA chunk
NT = 500                   # columns per matmul (fits one 2KB fp32 PSUM bank)
BF16 = np.dtype(ml_dtypes.bfloat16)

_cached_nc = {}


def _build(CH=CH, LOOK=3, EHB=7, OPB=6, MMBIG=False, WARM=True,
           PARTN=4, OUTLUMP=False, LDW1=True):
    key = (CH, LOOK, EHB, OPB, MMBIG, WARM, PARTN, OUTLUMP, LDW1)
    if key in _cached_nc:
        return _cached_nc[key]
    f32 = mybir.dt.float32
    bf16 = mybir.dt.bfloat16
    nc = bacc.Bacc("TRN2", target_bir_lowering=False, debug=False,
                   num_devices=NCORES)
    e_d = nc.dram_tensor("e_wv", (BPC, F, N), bf16, kind="ExternalInput").ap()
    h_d = nc.dram_tensor("h_w", (BPC, F, N), bf16, kind="ExternalInput").ap()
    # packed weights: [AT (128 cols) | BT (128 cols) | cT bitcast to bf16
    # pairs (2*BPC cols)] — one DMA with 520B contiguous lines instead of
    # three tiny transfers whose 256B/8B lines clog the sync ring for ~6us.
    WCOLS = 2 * F + 2 * BPC
    w_d = nc.dram_tensor("wpack", (F, WCOLS), bf16, kind="ExternalInput").ap()
    o_d = nc.dram_tensor("msg", (BPC, F, N), bf16, kind="ExternalOutput").ap()

    # chunk schedule per batch: big streaming chunks, tapered at the very
    # end so the pipeline drains with small PE/DMA quanta instead of one
    # full-size chunk of latency.
    def chunks_for(b):
        head = [CH]
        if WARM and b == 0:
            # small leading chunks so compute and the output stream start
            # ~3us earlier, filling both rings from the beginning
            head = [1000, 1000, CH - 2000]
        if b < BPC - 1:
            return head + [CH] * (N // CH - 1)
        taper, c = [], CH
        while c > 500:
            taper.append(c // 2)
            c -= c // 2
        taper.append(c)
        return head + [CH] * (N // CH - 2) + taper

    with tile.TileContext(nc) as tc:
        with tc.tile_pool(name="w", bufs=1) as wp, \
             tc.tile_pool(name="eh", bufs=EHB) as ehp, \
             tc.tile_pool(name="out", bufs=OPB) as opp, \
             tc.tile_pool(name="ps", bufs=8 // PARTN, space="PSUM") as psp:
            w_t = wp.tile([F, WCOLS], bf16)
            nc.sync.dma_start(w_t[:], w_d[:])
            at_t = w_t[:, 0:F]
            bt_t = w_t[:, F:2 * F]
            c_t = w_t[:, 2 * F:].bitcast(f32)       # [F, BPC] fp32 view

            sched = []
            for b in range(BPC):
                n0 = 0
                for cs in chunks_for(b):
                    sched.append((b, n0, cs))
                    n0 += cs
            tiles = {}

            def load(i):
                b, n0, cs = sched[i]
                e_t = ehp.tile([F, cs], bf16, tag="e")
                h_t = ehp.tile([F, cs], bf16, tag="h")
                nc.sync.dma_start(e_t[:], e_d[b, :, n0:n0 + cs])
                nc.scalar.dma_start(h_t[:], h_d[b, :, n0:n0 + cs])
                tiles[i] = (e_t, h_t)

            BANK = 512              # fp32 cols per 2KB PSUM bank
            ntail = len(sched) - 3  # taper chunks: outs go on sync/scalar

            def segs(p):
                # bank-aligned 512-col matmul segments; all but the last are
                # full banks, so the part's PSUM data stays contiguous
                out, off = [], 0
                while off < p:
                    w = min(BANK, p - off)
                    out.append((off, w))
                    off += w
                return out

            def compute_store(i):
                b, n0, cs = sched[i]
                e_t, h_t = tiles.pop(i)
                o_t = opp.tile([F, cs], bf16, tag="o")
                # two halves per chunk, each in a 4-bank PSUM tile so the
                # bias-add is ONE instruction per half; part0's bias-add runs
                # on the DVE, part1's concurrently on the ACT engine — a
                # single engine (~1.1ns/elem/lane) cannot keep up with the
                # 425 GB/s stream, especially under HW activity throttling
                PB = PARTN * BANK
                parts, plo = [], 0
                while plo < cs:
                    parts.append((plo, min(plo + PB, cs)))
                    plo = min(plo + PB, cs)
                ps_ts = []
                # LDW1: one standalone array weight-load per weight-group
                # per chunk; the matmuls then skip their fused reload (PE
                # program order keeps the array state valid). Halves PE
                # array occupancy lost to redundant 128-row weight loads.
                if LDW1:
                    nc.tensor.ldweights(at_t)
                for lo, hi in parts:
                    ps_t = psp.tile([F, PB], f32, tag="ps")
                    ps_ts.append(ps_t)
                    for off, w in (segs(hi - lo) if not MMBIG
                                   else [(0, hi - lo)]):
                        mi = nc.tensor.matmul(ps_t[:, off:off + w], at_t,
                                              e_t[:, lo + off:lo + off + w],
                                              start=True, stop=False)
                        if LDW1:
                            mi.ins.ldweights = False
                if LDW1:
                    nc.tensor.ldweights(bt_t)
                for pi, (lo, hi) in enumerate(parts):
                    p = hi - lo
                    ps_t = ps_ts[pi]
                    for off, w in (segs(p) if not MMBIG else [(0, p)]):
                        mi = nc.tensor.matmul(ps_t[:, off:off + w], bt_t,
                                              h_t[:, lo + off:lo + off + w],
                                              start=False, stop=True)
                        if LDW1:
                            mi.ins.ldweights = False
                    if (i + pi) % 2 == 0:
                        nc.vector.tensor_scalar_add(o_t[:, lo:hi],
                                                    ps_t[:, :p],
                                                    c_t[:, b:b + 1])
                    else:
                        nc.scalar.activation(
                            o_t[:, lo:hi], ps_t[:, :p],
                            mybir.ActivationFunctionType.Identity,
                            bias=c_t[:, b:b + 1])
                    # output halves split across the two HWDGE rings (byte
                    # balance), ring order alternating per chunk; SWDGE
                    # (gpsimd) outs were tried and lost ~9us — Q7 completion
                    # latency backs up the out-tile pool
                    rsel = i if OUTLUMP else i + pi
                    ring = nc.sync if rsel % 2 == 0 else nc.scalar
                    ring.dma_start(o_d[b, :, n0 + lo:n0 + hi],
                                   o_t[:, lo:hi])

            # software-pipelined trigger order: keep LOOK chunks of input
            # loads queued on each ring ahead of the compute/store triggers,
            # so an output trigger waiting on the DVE never starves the ring.
            for i in range(len(sched)):
                load(i)
                if i >= LOOK:
                    compute_store(i - LOOK)
            for i in range(len(sched) - LOOK, len(sched)):
                compute_store(i)
    nc.finalize()
    _cached_nc[key] = nc
    return nc


def _prepare_in_maps(h_w, h_v, e_wv, W_e2m, b_e2m, W_n2m, b_n2m,
                     W_resize, b_resize):
    f64 = np.float64
    M = F
    Wa = W_resize[:, :M].astype(f64)
    Wb = W_resize[:, M:2 * M].astype(f64)
    Wc = W_resize[:, 2 * M:].astype(f64)
    A = Wa @ W_e2m.astype(f64)
    Bm = Wb @ W_n2m.astype(f64)
    nv = h_v.astype(f64) @ W_n2m.astype(f64).T + b_n2m.astype(f64)
    c = (Wa @ b_e2m.astype(f64) + Wb @ b_n2m.astype(f64)
         + nv @ Wc.T + b_resize.astype(f64))          # [B, M]
    AT = np.ascontiguousarray(A.T).astype(BF16)
    BT = np.ascontiguousarray(Bm.T).astype(BF16)
    cT = np.ascontiguousarray(c.T).astype(np.float32)  # [M, B]

    e_bf = e_wv.astype(BF16)
    h_bf = h_w.astype(BF16)
    in_maps = []
    for cid in range(NCORES):
        bs = slice(cid * BPC, (cid + 1) * BPC)
        # pack [AT | BT | cT-bitcast] into one bf16 tensor (see _build)
        c_u16 = np.ascontiguousarray(cT[:, bs]).view(np.uint16)  # [F, 2*BPC]
        wpack = np.concatenate(
            [AT.view(np.uint16), BT.view(np.uint16), c_u16],
            axis=1).view(BF16)
        in_maps.append({
            "e_wv": np.ascontiguousarray(e_bf[bs]),
            "h_w": np.ascontiguousarray(h_bf[bs]),
            "wpack": np.ascontiguousarray(wpack),
        })
    return in_maps


def kernel(**inputs):
    args = {k: np.asarray(inputs[k], dtype=np.float32)
            for k in ("h_w", "h_v", "e_wv", "W_e2m", "b_e2m", "W_n2m",
                      "b_n2m", "W_resize", "b_resize")}
    in_maps = _prepare_in_maps(**args)
    nc = _build()
    res = run_bass_kernel_spmd(nc, in_maps, core_ids=list(range(NCORES)))
    return np.concatenate(
        [r["msg"].astype(np.float32) for r in res.results], axis=0)


# revision 28
# speedup vs baseline: 1.1414x; 1.1414x over previous
"""Trainium2 Bass kernel for nn_MessageFunctionForEvent (GNN message function).

Math: the reference is
    em  = W_e2m @ e_wv[b] + b_e2m          (per-node Linear on edge features)
    nw  = W_n2m @ h_w[b]  + b_n2m          (per-node Linear on node features)
    nv  = W_n2m @ h_v[b]  + b_n2m          (node-level, no n axis)
    msg = Wa @ em + Wb @ nw + (Wc @ nv + b_resize)[:, None]
which collapses (precomposing the tiny 128x128 weights on host) to
    msg[b, :, n] = A @ e_wv[b, :, n] + Bm @ h_w[b, :, n] + c[b]
with A = Wa@W_e2m, Bm = Wb@W_n2m, c[b] = Wa@b_e2m + Wb@b_n2m + Wc@nv[b] + b_resize.

The problem is HBM-bound (per-core traffic >> compute), so the streams are
cast to bf16 on the host: e/h chunks and the two 128x128 weights go over the
wire in bf16, matmuls accumulate in fp32 PSUM, the bias-add writes a bf16
output tile, and the host upcasts the result to fp32. This halves HBM traffic
(61.4MB -> 30.7MB per core) for ~1.3e-3 normed rel error (gate is 2e-2).

Device kernel: a single HWDGE ring tops out ~385 GB/s but both rings
together sustain ~425 GB/s (the SBUF AXI fabric limit), so the two rings
are byte-balanced end-to-end: e chunks on the sync(SP) ring, h chunks on
the scalar(ACT) ring, and each chunk's two output halves split across
both rings (ring order alternating per chunk). The packed precomposed
weights (AT|BT|c bit-cast into one bf16 tensor, 520B lines) go over the
sync ring first so compute unblocks immediately; three separate small
weight DMAs would clog the ring for ~6us with sub-512B descriptors.
Per chunk: 512-col bank-aligned matmul segments accumulate A@e then B@h
into two 4-bank fp32 PSUM tiles (all A-segments first — they only need
e — then B as h lands); each half-chunk gets ONE bias-add instruction
(PSUM fp32 -> SBUF bf16), part0 on the VectorE, part1 concurrently on
the ACT engine — one engine alone (~1.1ns/elem/lane) cannot keep up,
especially under HW activity throttling. Load triggers run LOOK=3
chunks ahead of compute/store triggers so an output trigger waiting on
its bias-add never starves a ring (deeper lookahead queues outputs
behind too many input transfers in the ring FIFO and blows up the
drain). The first chunk is split small so the output stream starts
~3us earlier, and the last chunk tapers down so the pipeline drains
with small quanta. Sharding: batch axis (16 batches -> 2 per core).
"""

import sys

import numpy as np
import ml_dtypes

try:
    from concourse import bacc, mybir
except ImportError:  # bare environment: fall back to the in-container repo
    sys.path.append("/opt/trn_rl_repo")
    from concourse import bacc, mybir
import concourse.tile as tile
from concourse.bass_utils import run_bass_kernel_spmd

B, F, N = 16, 128, 20000
NCORES = 8
BPC = B // NCORES          # batches per core
CH = 4000                  # columns per DMA chunk
NT = 500                   # columns per matmul (fits one 2KB fp32 PSUM bank)
BF16 = np.dtype(ml_dtypes.bfloat16)

_cached_nc = {}


def _build(CH=CH, LOOK=3, EHB=7, OPB=6, MMBIG=False, WARM=True,
           PARTN=4, OUTLUMP=False, LDW1=True, OUTMODE=1):
    key = (CH, LOOK, EHB, OPB, MMBIG, WARM, PARTN, OUTLUMP, LDW1, OUTMODE)
    if key in _cached_nc:
        return _cached_nc[key]
    f32 = mybir.dt.float32
    bf16 = mybir.dt.bfloat16
    nc = bacc.Bacc("TRN2", target_bir_lowering=False, debug=False,
                   num_devices=NCORES)
    e_d = nc.dram_tensor("e_wv", (BPC, F, N), bf16, kind="ExternalInput").ap()
    h_d = nc.dram_tensor("h_w", (BPC, F, N), bf16, kind="ExternalInput").ap()
    # packed weights: [AT (128 cols) | BT (128 cols) | cT bitcast to bf16
    # pairs (2*BPC cols)] — one DMA with 520B contiguous lines instead of
    # three tiny transfers whose 256B/8B lines clog the sync ring for ~6us.
    WCOLS = 2 * F + 2 * BPC
    w_d = nc.dram_tensor("wpack", (F, WCOLS), bf16, kind="ExternalInput").ap()
    o_d = nc.dram_tensor("msg", (BPC, F, N), bf16, kind="ExternalOutput").ap()

    # chunk schedule per batch: big streaming chunks, tapered at the very
    # end so the pipeline drains with small PE/DMA quanta instead of one
    # full-size chunk of latency.
    def chunks_for(b):
        head = [CH]
        if WARM and b == 0:
            # small leading chunks so compute and the output stream start
            # ~3us earlier, filling both rings from the beginning
            head = [1000, 1000, CH - 2000]
        if b < BPC - 1:
            return head + [CH] * (N // CH - 1)
        taper, c = [], CH
        while c > 500:
            taper.append(c // 2)
            c -= c // 2
        taper.append(c)
        return head + [CH] * (N // CH - 2) + taper

    with tile.TileContext(nc) as tc:
        with tc.tile_pool(name="w", bufs=1) as wp, \
             tc.tile_pool(name="eh", bufs=EHB) as ehp, \
             tc.tile_pool(name="out", bufs=OPB) as opp, \
             tc.tile_pool(name="ps", bufs=8 // PARTN, space="PSUM") as psp:
            w_t = wp.tile([F, WCOLS], bf16)
            nc.sync.dma_start(w_t[:], w_d[:])
            at_t = w_t[:, 0:F]
            bt_t = w_t[:, F:2 * F]
            c_t = w_t[:, 2 * F:].bitcast(f32)       # [F, BPC] fp32 view

            sched = []
            for b in range(BPC):
                n0 = 0
                for cs in chunks_for(b):
                    sched.append((b, n0, cs))
                    n0 += cs
            tiles = {}

            def load(i):
                b, n0, cs = sched[i]
                e_t = ehp.tile([F, cs], bf16, tag="e")
                h_t = ehp.tile([F, cs], bf16, tag="h")
                nc.sync.dma_start(e_t[:], e_d[b, :, n0:n0 + cs])
                nc.scalar.dma_start(h_t[:], h_d[b, :, n0:n0 + cs])
                tiles[i] = (e_t, h_t)

            BANK = 512              # fp32 cols per 2KB PSUM bank
            ntail = len(sched) - 3  # taper chunks: outs go on sync/scalar

            def segs(p):
                # bank-aligned 512-col matmul segments; all but the last are
                # full banks, so the part's PSUM data stays contiguous
                out, off = [], 0
                while off < p:
                    w = min(BANK, p - off)
                    out.append((off, w))
                    off += w
                return out

            def compute_store(i):
                b, n0, cs = sched[i]
                e_t, h_t = tiles.pop(i)
                o_t = opp.tile([F, cs], bf16, tag="o")
                # two halves per chunk, each in a 4-bank PSUM tile so the
                # bias-add is ONE instruction per half; part0's bias-add runs
                # on the DVE, part1's concurrently on the ACT engine — a
                # single engine (~1.1ns/elem/lane) cannot keep up with the
                # 425 GB/s stream, especially under HW activity throttling
                PB = PARTN * BANK
                parts, plo = [], 0
                while plo < cs:
                    parts.append((plo, min(plo + PB, cs)))
                    plo = min(plo + PB, cs)
                ps_ts = []
                # LDW1: one standalone array weight-load per weight-group
                # per chunk; the matmuls then skip their fused reload (PE
                # program order keeps the array state valid). Halves PE
                # array occupancy lost to redundant 128-row weight loads.
                if LDW1:
                    nc.tensor.ldweights(at_t)
                for lo, hi in parts:
                    ps_t = psp.tile([F, PB], f32, tag="ps")
                    ps_ts.append(ps_t)
                    for off, w in (segs(hi - lo) if not MMBIG
                                   else [(0, hi - lo)]):
                        mi = nc.tensor.matmul(ps_t[:, off:off + w], at_t,
                                              e_t[:, lo + off:lo + off + w],
                                              start=True, stop=False)
                        if LDW1:
                            mi.ins.ldweights = False
                if LDW1:
                    nc.tensor.ldweights(bt_t)
                for pi, (lo, hi) in enumerate(parts):
                    p = hi - lo
                    ps_t = ps_ts[pi]
                    for off, w in (segs(p) if not MMBIG else [(0, p)]):
                        mi = nc.tensor.matmul(ps_t[:, off:off + w], bt_t,
                                              h_t[:, lo + off:lo + off + w],
                                              start=False, stop=True)
                        if LDW1:
                            mi.ins.ldweights = False
                    if (i + pi) % 2 == 0:
                        nc.vector.tensor_scalar_add(o_t[:, lo:hi],
                                                    ps_t[:, :p],
                                                    c_t[:, b:b + 1])
                    else:
                        nc.scalar.activation(
                            o_t[:, lo:hi], ps_t[:, :p],
                            mybir.ActivationFunctionType.Identity,
                            bias=c_t[:, b:b + 1])
                    # output halves split across the two HWDGE rings (byte
                    # balance), ring order alternating per chunk; SWDGE
                    # (gpsimd) outs were tried and lost ~9us — Q7 completion
                    # latency backs up the out-tile pool
                    if OUTMODE == 1:      # both outs on sync: ACT seq relief
                        ring = nc.sync
                    elif OUTMODE == 2:    # out1 via SWDGE; taper outs HWDGE
                        ring = (nc.sync if pi % 2 == 0 else
                                (nc.gpsimd if i < ntail else nc.scalar))
                    else:
                        rsel = i if OUTLUMP else i + pi
                        ring = nc.sync if rsel % 2 == 0 else nc.scalar
                    ring.dma_start(o_d[b, :, n0 + lo:n0 + hi],
                                   o_t[:, lo:hi])

            # software-pipelined trigger order: keep LOOK chunks of input
            # loads queued on each ring ahead of the compute/store triggers,
            # so an output trigger waiting on the DVE never starves the ring.
            for i in range(len(sched)):
                load(i)
                if i >= LOOK:
                    compute_store(i - LOOK)
            for i in range(len(sched) - LOOK, len(sched)):
                compute_store(i)
    nc.finalize()
    _cached_nc[key] = nc
    return nc


def _prepare_in_maps(h_w, h_v, e_wv, W_e2m, b_e2m, W_n2m, b_n2m,
                     W_resize, b_resize):
    f64 = np.float64
    M = F
    Wa = W_resize[:, :M].astype(f64)
    Wb = W_resize[:, M:2 * M].astype(f64)
    Wc = W_resize[:, 2 * M:].astype(f64)
    A = Wa @ W_e2m.astype(f64)
    Bm = Wb @ W_n2m.astype(f64)
    nv = h_v.astype(f64) @ W_n2m.astype(f64).T + b_n2m.astype(f64)
    c = (Wa @ b_e2m.astype(f64) + Wb @ b_n2m.astype(f64)
         + nv @ Wc.T + b_resize.astype(f64))          # [B, M]
    AT = np.ascontiguousarray(A.T).astype(BF16)
    BT = np.ascontiguousarray(Bm.T).astype(BF16)
    cT = np.ascontiguousarray(c.T).astype(np.float32)  # [M, B]

    e_bf = e_wv.astype(BF16)
    h_bf = h_w.astype(BF16)
    in_maps = []
    for cid in range(NCORES):
        bs = slice(cid * BPC, (cid + 1) * BPC)
        # pack [AT | BT | cT-bitcast] into one bf16 tensor (see _build)
        c_u16 = np.ascontiguousarray(cT[:, bs]).view(np.uint16)  # [F, 2*BPC]
        wpack = np.concatenate(
            [AT.view(np.uint16), BT.view(np.uint16), c_u16],
            axis=1).view(BF16)
        in_maps.append({
            "e_wv": np.ascontiguousarray(e_bf[bs]),
            "h_w": np.ascontiguousarray(h_bf[bs]),
            "wpack": np.ascontiguousarray(wpack),
        })
    return in_maps


def kernel(**inputs):
    args = {k: np.asarray(inputs[k], dtype=np.float32)
            for k in ("h_w", "h_v", "e_wv", "W_e2m", "b_e2m", "W_n2m",
                      "b_n2m", "W_resize", "b_resize")}
    in_maps = _prepare_in_maps(**args)
    nc = _build()
    res = run_bass_kernel_spmd(nc, in_maps, core_ids=list(range(NCORES)))
    return np.concatenate(
        [r["msg"].astype(np.float32) for r in res.results], axis=0)


# revision 29
# speedup vs baseline: 1.1574x; 1.0141x over previous
"""Trainium2 Bass kernel for nn_MessageFunctionForEvent (GNN message function).

Math: the reference is
    em  = W_e2m @ e_wv[b] + b_e2m          (per-node Linear on edge features)
    nw  = W_n2m @ h_w[b]  + b_n2m          (per-node Linear on node features)
    nv  = W_n2m @ h_v[b]  + b_n2m          (node-level, no n axis)
    msg = Wa @ em + Wb @ nw + (Wc @ nv + b_resize)[:, None]
which collapses (precomposing the tiny 128x128 weights on host) to
    msg[b, :, n] = A @ e_wv[b, :, n] + Bm @ h_w[b, :, n] + c[b]
with A = Wa@W_e2m, Bm = Wb@W_n2m, c[b] = Wa@b_e2m + Wb@b_n2m + Wc@nv[b] + b_resize.

The problem is HBM-bound (per-core traffic >> compute), so the streams are
cast to bf16 on the host: e/h chunks and the two 128x128 weights go over the
wire in bf16, matmuls accumulate in fp32 PSUM, the bias-add writes a bf16
output tile, and the host upcasts the result to fp32. This halves HBM traffic
(61.4MB -> 30.7MB per core) for ~1.3e-3 normed rel error (gate is 2e-2).

Device kernel: a single HWDGE ring tops out ~385 GB/s but both rings
together sustain ~425 GB/s (the SBUF AXI fabric limit), so the two rings
are byte-balanced end-to-end: e chunks on the sync(SP) ring, h chunks on
the scalar(ACT) ring, and each chunk's two output halves split across
both rings (ring order alternating per chunk). The packed precomposed
weights (AT|BT|c bit-cast into one bf16 tensor, 520B lines) go over the
sync ring first so compute unblocks immediately; three separate small
weight DMAs would clog the ring for ~6us with sub-512B descriptors.
Per chunk: 512-col bank-aligned matmul segments accumulate A@e then B@h
into two 4-bank fp32 PSUM tiles (all A-segments first — they only need
e — then B as h lands); each half-chunk gets ONE bias-add instruction
(PSUM fp32 -> SBUF bf16), part0 on the VectorE, part1 concurrently on
the ACT engine — one engine alone (~1.1ns/elem/lane) cannot keep up,
especially under HW activity throttling. Load triggers run LOOK=3
chunks ahead of compute/store triggers so an output trigger waiting on
its bias-add never starves a ring (deeper lookahead queues outputs
behind too many input transfers in the ring FIFO and blows up the
drain). The first chunk is split small so the output stream starts
~3us earlier, and the last chunk tapers down so the pipeline drains
with small quanta. Sharding: batch axis (16 batches -> 2 per core).
"""

import sys

import numpy as np
import ml_dtypes

try:
    from concourse import bacc, mybir
except ImportError:  # bare environment: fall back to the in-container repo
    sys.path.append("/opt/trn_rl_repo")
    from concourse import bacc, mybir
import concourse.tile as tile
from concourse.bass_utils import run_bass_kernel_spmd

B, F, N = 16, 128, 20000
NCORES = 8
BPC = B // NCORES          # batches per core
CH = 4000                  # columns per DMA chunk
NT = 500                   # columns per matmul (fits one 2KB fp32 PSUM bank)
BF16 = np.dtype(ml_dtypes.bfloat16)

_cached_nc = {}


def _build(CH=CH, LOOK=3, EHB=9, OPB=6, MMBIG=False, WARM=True,
           PARTN=4, OUTLUMP=False, LDW1=True, OUTMODE=1):
    key = (CH, LOOK, EHB, OPB, MMBIG, WARM, PARTN, OUTLUMP, LDW1, OUTMODE)
    if key in _cached_nc:
        return _cached_nc[key]
    f32 = mybir.dt.float32
    bf16 = mybir.dt.bfloat16
    nc = bacc.Bacc("TRN2", target_bir_lowering=False, debug=False,
                   num_devices=NCORES)
    e_d = nc.dram_tensor("e_wv", (BPC, F, N), bf16, kind="ExternalInput").ap()
    h_d = nc.dram_tensor("h_w", (BPC, F, N), bf16, kind="ExternalInput").ap()
    # packed weights: [AT (128 cols) | BT (128 cols) | cT bitcast to bf16
    # pairs (2*BPC cols)] — one DMA with 520B contiguous lines instead of
    # three tiny transfers whose 256B/8B lines clog the sync ring for ~6us.
    WCOLS = 2 * F + 2 * BPC
    w_d = nc.dram_tensor("wpack", (F, WCOLS), bf16, kind="ExternalInput").ap()
    o_d = nc.dram_tensor("msg", (BPC, F, N), bf16, kind="ExternalOutput").ap()

    # chunk schedule per batch: big streaming chunks, tapered at the very
    # end so the pipeline drains with small PE/DMA quanta instead of one
    # full-size chunk of latency.
    def chunks_for(b):
        head = [CH]
        if WARM and b == 0:
            # small leading chunks so compute and the output stream start
            # ~3us earlier, filling both rings from the beginning
            head = [1000, 1000, CH - 2000]
        if b < BPC - 1:
            return head + [CH] * (N // CH - 1)
        taper, c = [], CH
        while c > 500:
            taper.append(c // 2)
            c -= c // 2
        taper.append(c)
        return head + [CH] * (N // CH - 2) + taper

    with tile.TileContext(nc) as tc:
        with tc.tile_pool(name="w", bufs=1) as wp, \
             tc.tile_pool(name="eh", bufs=EHB) as ehp, \
             tc.tile_pool(name="out", bufs=OPB) as opp, \
             tc.tile_pool(name="ps", bufs=8 // PARTN, space="PSUM") as psp:
            w_t = wp.tile([F, WCOLS], bf16)
            nc.sync.dma_start(w_t[:], w_d[:])
            at_t = w_t[:, 0:F]
            bt_t = w_t[:, F:2 * F]
            c_t = w_t[:, 2 * F:].bitcast(f32)       # [F, BPC] fp32 view

            sched = []
            for b in range(BPC):
                n0 = 0
                for cs in chunks_for(b):
                    sched.append((b, n0, cs))
                    n0 += cs
            tiles = {}

            def load(i):
                b, n0, cs = sched[i]
                e_t = ehp.tile([F, cs], bf16, tag="e")
                h_t = ehp.tile([F, cs], bf16, tag="h")
                nc.sync.dma_start(e_t[:], e_d[b, :, n0:n0 + cs])
                nc.scalar.dma_start(h_t[:], h_d[b, :, n0:n0 + cs])
                tiles[i] = (e_t, h_t)

            BANK = 512              # fp32 cols per 2KB PSUM bank
            ntail = len(sched) - 3  # taper chunks: outs go on sync/scalar

            def segs(p):
                # bank-aligned 512-col matmul segments; all but the last are
                # full banks, so the part's PSUM data stays contiguous
                out, off = [], 0
                while off < p:
                    w = min(BANK, p - off)
                    out.append((off, w))
                    off += w
                return out

            def compute_store(i):
                b, n0, cs = sched[i]
                e_t, h_t = tiles.pop(i)
                o_t = opp.tile([F, cs], bf16, tag="o")
                # two halves per chunk, each in a 4-bank PSUM tile so the
                # bias-add is ONE instruction per half; part0's bias-add runs
                # on the DVE, part1's concurrently on the ACT engine — a
                # single engine (~1.1ns/elem/lane) cannot keep up with the
                # 425 GB/s stream, especially under HW activity throttling
                PB = PARTN * BANK
                parts, plo = [], 0
                while plo < cs:
                    parts.append((plo, min(plo + PB, cs)))
                    plo = min(plo + PB, cs)
                ps_ts = []
                # LDW1: one standalone array weight-load per weight-group
                # per chunk; the matmuls then skip their fused reload (PE
                # program order keeps the array state valid). Halves PE
                # array occupancy lost to redundant 128-row weight loads.
                if LDW1:
                    nc.tensor.ldweights(at_t)
                for lo, hi in parts:
                    ps_t = psp.tile([F, PB], f32, tag="ps")
                    ps_ts.append(ps_t)
                    for off, w in (segs(hi - lo) if not MMBIG
                                   else [(0, hi - lo)]):
                        mi = nc.tensor.matmul(ps_t[:, off:off + w], at_t,
                                              e_t[:, lo + off:lo + off + w],
                                              start=True, stop=False)
                        if LDW1:
                            mi.ins.ldweights = False
                if LDW1:
                    nc.tensor.ldweights(bt_t)
                for pi, (lo, hi) in enumerate(parts):
                    p = hi - lo
                    ps_t = ps_ts[pi]
                    for off, w in (segs(p) if not MMBIG else [(0, p)]):
                        mi = nc.tensor.matmul(ps_t[:, off:off + w], bt_t,
                                              h_t[:, lo + off:lo + off + w],
                                              start=False, stop=True)
                        if LDW1:
                            mi.ins.ldweights = False
                    if (i + pi) % 2 == 0:
                        nc.vector.tensor_scalar_add(o_t[:, lo:hi],
                                                    ps_t[:, :p],
                                                    c_t[:, b:b + 1])
                    else:
                        nc.scalar.activation(
                            o_t[:, lo:hi], ps_t[:, :p],
                            mybir.ActivationFunctionType.Identity,
                            bias=c_t[:, b:b + 1])
                    # output halves split across the two HWDGE rings (byte
                    # balance), ring order alternating per chunk; SWDGE
                    # (gpsimd) outs were tried and lost ~9us — Q7 completion
                    # latency backs up the out-tile pool
                    if OUTMODE == 1:      # both outs on sync: ACT seq relief
                        ring = nc.sync
                    elif OUTMODE == 2:    # out1 via SWDGE; taper outs HWDGE
                        ring = (nc.sync if pi % 2 == 0 else
                                (nc.gpsimd if i < ntail else nc.scalar))
                    else:
                        rsel = i if OUTLUMP else i + pi
                        ring = nc.sync if rsel % 2 == 0 else nc.scalar
                    ring.dma_start(o_d[b, :, n0 + lo:n0 + hi],
                                   o_t[:, lo:hi])

            # software-pipelined trigger order: keep LOOK chunks of input
            # loads queued on each ring ahead of the compute/store triggers,
            # so an output trigger waiting on the DVE never starves the ring.
            for i in range(len(sched)):
                load(i)
                if i >= LOOK:
                    compute_store(i - LOOK)
            for i in range(len(sched) - LOOK, len(sched)):
                compute_store(i)
    nc.finalize()
    _cached_nc[key] = nc
    return nc


def _prepare_in_maps(h_w, h_v, e_wv, W_e2m, b_e2m, W_n2m, b_n2m,
                     W_resize, b_resize):
    f64 = np.float64
    M = F
    Wa = W_resize[:, :M].astype(f64)
    Wb = W_resize[:, M:2 * M].astype(f64)
    Wc = W_resize[:, 2 * M:].astype(f64)
    A = Wa @ W_e2m.astype(f64)
    Bm = Wb @ W_n2m.astype(f64)
    nv = h_v.astype(f64) @ W_n2m.astype(f64).T + b_n2m.astype(f64)
    c = (Wa @ b_e2m.astype(f64) + Wb @ b_n2m.astype(f64)
         + nv @ Wc.T + b_resize.astype(f64))          # [B, M]
    AT = np.ascontiguousarray(A.T).astype(BF16)
    BT = np.ascontiguousarray(Bm.T).astype(BF16)
    cT = np.ascontiguousarray(c.T).astype(np.float32)  # [M, B]

    e_bf = e_wv.astype(BF16)
    h_bf = h_w.astype(BF16)
    in_maps = []
    for cid in range(NCORES):
        bs = slice(cid * BPC, (cid + 1) * BPC)
        # pack [AT | BT | cT-bitcast] into one bf16 tensor (see _build)
        c_u16 = np.ascontiguousarray(cT[:, bs]).view(np.uint16)  # [F, 2*BPC]
        wpack = np.concatenate(
            [AT.view(np.uint16), BT.view(np.uint16), c_u16],
            axis=1).view(BF16)
        in_maps.append({
            "e_wv": np.ascontiguousarray(e_bf[bs]),
            "h_w": np.ascontiguousarray(h_bf[bs]),
            "wpack": np.ascontiguousarray(wpack),
        })
    return in_maps


def kernel(**inputs):
    args = {k: np.asarray(inputs[k], dtype=np.float32)
            for k in ("h_w", "h_v", "e_wv", "W_e2m", "b_e2m", "W_n2m",
                      "b_n2m", "W_resize", "b_resize")}
    in_maps = _prepare_in_maps(**args)
    nc = _build()
    res = run_bass_kernel_spmd(nc, in_maps, core_ids=list(range(NCORES)))
    return np.concatenate(
        [r["msg"].astype(np.float32) for r in res.results], axis=0)
